# revision 1
# baseline (speedup 1.0000x reference)
"""Bbox regression loss (smooth-L1 over gathered bbox deltas) on 8 TRN2 cores.

The loss gathers 4 scalars per (batch, gt-box) from each FPN level's dense
prediction tensor, applies smooth-L1 against the gt deltas, and reduces to
two scalars (weighted loss sum, valid-box count).  Only 3 x 2 x 128 x 4 =
3072 elements of the ~92MB of predictions are ever read, so the kernel is
built around one on-device dma_gather rather than streaming.

Sharding: core c handles (b = c//4, k = c%4) where k indexes the 4 bbox
coordinate channels (channel group k*A:(k+1)*A of the 4*A=12 channel dim).
Each core receives exactly 1/8 of every prediction tensor (concatenated
into one row table), computes its partial (loss, weight) fully on device,
and the host sums the 8 partials.

Device pipeline per core:
  1. coord math -> 512B-row indices for all 3 levels in the
     16-partition-wrapped int16 layout dma_gather wants.  The host ships
     stride-premultiplied coordinate terms (incl. the concat-table row
     base as a 5th term), so the device only clamps, 5-term-reduces and
     shifts.  Clamping the premultiplied anchor term is exact because only
     the anchor carries the -1 sentinel and the strides are positive.
  2. one dma_gather fetches the 384 rows -> [128(m), 3(level), 128] f32
  3. fused scalar_tensor_tensor one-hot select (iota==rem)*g with
     per-partition accumulate -> pred[m,l]
  4. smooth-L1 via the huber identity 0.5*(d^2 - relu(|d|-1)^2) (the 0.5
     folded into the host-side weight), validity masking, one matmul
     partition-reduction, per-level active mask applied on partition 0
"""

import os

import numpy as np

try:  # persistent XLA/NEFF compile cache across processes
    import jax

    os.makedirs("/tmp/jax_pcache", exist_ok=True)
    jax.config.update("jax_compilation_cache_dir", "/tmp/jax_pcache")
    jax.config.update("jax_persistent_cache_min_compile_time_secs", 0.0)
    jax.config.update("jax_persistent_cache_min_entry_size_bytes", 0)
except Exception:
    pass

import concourse.bacc as bacc
import concourse.bass as bass
import concourse.tile as tile
from concourse import mybir
from concourse.bass_utils import run_bass_kernel_spmd

A = 3                       # anchors per level
M = 128                     # gt entries per sample
GRIDS = (96, 48, 24)        # level l grid; level l uses coord/diff index 2-l
LOSS_W = (1.0, 1.0, 1.0, 0.1)
ROW = 128                   # f32 elements per gather row (512B)
NLVL = 3
NIDX = NLVL * M             # 384 gathered rows per core
V = tuple(A * g * g * g // ROW for g in GRIDS)      # (20736, 2592, 324)
VBASE = (0, V[0], V[0] + V[1])
VTOT = sum(V)               # 23652 rows < int16 max
N_CORES = 8

F32 = mybir.dt.float32
I32 = mybir.dt.int32
I16 = mybir.dt.int16
Alu = mybir.AluOpType

# auxw (int32, [128, 128]): stride-premultiplied coord terms, 24 groups x 4
#   [max(a,0)*g^3, d*g^2, h*g, w + VBASE[l]*ROW], wrapped layout (l,q):
#   partition = m%16, m = q*16+p%16.  The anchor clamp (the reference's
#   jnp.maximum(c,0) gather guard) is applied host-side here; validity is
#   still derived on device from the unclamped natural-layout anchor term.
# auxn (int32, [128, 12]): same 4 terms, natural layout (partition = m),
#   anchor term UNclamped (carries the -1 sentinel for the validity test).
NWRAP = 128  # 96 used + 32 pad cols to reach 512B/partition (full-rate DMA)
NNAT = 12
# auxf (f32, [128, 132]): iota128 | gts(3) | ones
NF_COLS = ROW + 4


def _build_bass() -> bass.Bass:
    nc = bacc.Bacc(
        "TRN2", target_bir_lowering=False, debug=False, num_devices=N_CORES
    )
    tab = nc.dram_tensor("tab", [VTOT, ROW], F32, kind="ExternalInput")
    auxw = nc.dram_tensor("auxw", [M, NWRAP], I32, kind="ExternalInput")
    auxn = nc.dram_tensor("auxn", [M, NNAT], I32, kind="ExternalInput")
    auxf = nc.dram_tensor("auxf", [M, NF_COLS], F32, kind="ExternalInput")
    out = nc.dram_tensor("partial", [1, 6], F32, kind="ExternalOutput")

    with tile.TileContext(nc) as tc:
        with (
            tc.tile_pool(name="sb", bufs=1) as sb,
            tc.tile_pool(name="ps", bufs=1, space="PSUM") as ps,
        ):
            tw = sb.tile([M, NWRAP], I32)
            nc.sync.dma_start(out=tw[:], in_=auxw[:])
            tn = sb.tile([M, NNAT], I32)
            nc.sync.dma_start(out=tn[:], in_=auxn[:])
            tf = sb.tile([M, NF_COLS], F32)
            nc.sync.dma_start(out=tf[:], in_=auxf[:])
            iota = tf[:, 0:ROW]
            gts = tf[:, ROW : ROW + 3]
            onec = tf[:, ROW + 3 : ROW + 4]

            # flatw[., c] = sum of premultiplied terms (anchor pre-clamped)
            flatw = sb.tile([M, 24], I32)
            with nc.allow_low_precision(reason="exact int32 index arithmetic"):
                nc.vector.tensor_reduce(
                    flatw[:],
                    tw[:, 0:96].rearrange("p (c f) -> p c f", f=4),
                    axis=mybir.AxisListType.X,
                    op=Alu.add,
                )

            # wrapped row indices: row = flatw >> 7 (bitwise can't cast;
            # the max-0 no-op does the int32->int16 conversion)
            rowi = sb.tile([M, 24], I32)
            nc.vector.tensor_scalar(
                rowi[:], flatw[:], 7, None, Alu.arith_shift_right
            )
            idx16 = sb.tile([M, 24], I16)
            nc.vector.tensor_scalar(idx16[:], rowi[:], 0, None, Alu.max)

            # one dma_gather for all 384 rows: g[m, l, :] = tab[idx(m,l), :]
            g = sb.tile([M, NLVL, ROW], F32)
            nc.gpsimd.dma_gather(g[:], tab[:], idx16[:], NIDX, NIDX, ROW)

            # natural-layout remainder + validity (runs under the gather)
            flatn = sb.tile([M, 3], I32)
            with nc.allow_low_precision(reason="exact int32 index arithmetic"):
                nc.vector.tensor_reduce(
                    flatn[:],
                    tn[:].rearrange("p (c f) -> p c f", f=4),
                    axis=mybir.AxisListType.X,
                    op=Alu.add,
                )
            rem = sb.tile([M, 3], I32)
            nc.vector.tensor_scalar(
                rem[:], flatn[:], ROW - 1, None, Alu.bitwise_and
            )
            remf = sb.tile([M, 3], F32)
            nc.vector.tensor_copy(remf[:], rem[:])
            combo = sb.tile([M, 6], F32)
            validf = combo[:, 3:6]
            anchors = tn[:].rearrange("p (l f) -> p l f", f=4)[:, :, 0:1]
            nc.vector.tensor_scalar(
                validf.rearrange("p (l f) -> p l f", f=1),
                anchors,
                -1,
                None,
                Alu.is_gt,
            )
            # pred[m,l] = g[m,l,rem[m,l]] -- fused (iota==rem)*g + row-sum
            pred = sb.tile([M, 3], F32)
            scratch = sb.tile([M, ROW], F32)
            for l in range(3):
                nc.vector.scalar_tensor_tensor(
                    out=scratch[:],
                    in0=iota,
                    scalar=remf[:, l : l + 1],
                    in1=g[:, l, :],
                    op0=Alu.is_equal,
                    op1=Alu.mult,
                    accum_out=pred[:, l : l + 1],
                )

            # smooth l1 (x2): d^2 - relu(|d|-1)^2   (the 0.5 lives in wk)
            d = sb.tile([M, 3], F32)
            nc.vector.tensor_tensor(d[:], pred[:], gts, Alu.subtract)
            dd = sb.tile([M, 3], F32)
            nc.vector.tensor_tensor(dd[:], d[:], d[:], Alu.mult)
            nd = sb.tile([M, 3], F32)
            nc.vector.tensor_scalar(nd[:], d[:], -1.0, None, Alu.mult)
            ad = sb.tile([M, 3], F32)
            nc.vector.tensor_tensor(ad[:], d[:], nd[:], Alu.max)
            t = sb.tile([M, 3], F32)
            nc.vector.tensor_scalar(t[:], ad[:], 1.0, 0.0, Alu.subtract, Alu.max)
            tt2 = sb.tile([M, 3], F32)
            nc.vector.tensor_tensor(tt2[:], t[:], t[:], Alu.mult)
            sl2 = sb.tile([M, 3], F32)
            nc.vector.tensor_tensor(sl2[:], dd[:], tt2[:], Alu.subtract)

            # combo = [ sl2*valid | valid ] -> one matmul -> [1,6];
            # wk/wen and the per-level active mask applied on partition 0
            nc.vector.tensor_tensor(combo[:, 0:3], sl2[:], validf, Alu.mult)
            pt6 = ps.tile([1, 6], F32)
            nc.tensor.matmul(
                out=pt6[:], lhsT=onec, rhs=combo[:], start=True, stop=True
            )
            res6 = sb.tile([1, 6], F32)
            act_b = (
                combo[0:1, 3:6]
                .rearrange("p (a l) -> p a l", a=1)
                .broadcast_to([1, 2, 3])
            )
            nc.vector.tensor_tensor(
                res6[:].rearrange("p (a l) -> p a l", l=3),
                pt6[:].rearrange("p (a l) -> p a l", l=3),
                act_b,
                Alu.mult,
            )
            nc.sync.dma_start(out=out[:], in_=res6[:])
    nc.finalize()
    return nc


_NC = None


def _get_nc():
    global _NC
    if _NC is None:
        _NC = _build_bass()
    return _NC


_IOTA = np.tile(np.arange(ROW, dtype=np.float32), (M, 1))
_STRIDE5 = {
    g: np.array([g**3, g**2, g, 1, 1], dtype=np.int64) for g in GRIDS
}


def kernel(**inputs: np.ndarray):
    out_l = [np.asarray(inputs[n]) for n in ("out1", "out3", "out5")]
    # level l uses coord/diff (2-l)  (the reference pairs them reversed)
    coords = [np.asarray(inputs[f"coord{2 - l}"]) for l in range(3)]
    diffs = [np.asarray(inputs[f"diff{2 - l}"]) for l in range(3)]

    in_maps = []
    for c in range(N_CORES):
        b, k = c // 4, c % 4
        im = {}
        im["tab"] = np.concatenate(
            [
                np.ascontiguousarray(out_l[l][b, A * k : A * (k + 1)]).reshape(
                    V[l], ROW
                )
                for l in range(3)
            ],
            axis=0,
        )
        cow = np.zeros((M, NWRAP), np.int32)
        con = np.zeros((M, NNAT), np.int32)
        for l, g in enumerate(GRIDS):
            s = _STRIDE5[g]
            cc = coords[l][b].astype(np.int64)  # [128, 4]
            # 4 premultiplied terms; table row base folded into the w term
            p4 = cc * s[:4]
            p4[:, 3] += VBASE[l] * ROW
            con[:, l * 4 : (l + 1) * 4] = p4.astype(np.int32)
            p4c = p4.copy()
            p4c[:, 0] = np.maximum(cc[:, 0], 0) * s[0]  # anchor gather clamp
            w = (
                p4c.astype(np.int32).reshape(8, 16, 4).transpose(1, 0, 2)
            ).reshape(16, 32)
            cow[:, l * 32 : (l + 1) * 32] = np.tile(w, (8, 1))
        im["auxw"] = cow
        im["auxn"] = con
        gts = np.stack([diffs[l][b, :, k] for l in range(3)], axis=1)
        onesc = np.ones((M, 1), np.float32)
        im["auxf"] = np.concatenate([_IOTA, gts, onesc], axis=1).astype(np.float32)
        in_maps.append(im)

    global _last_in_maps
    _last_in_maps = in_maps
    res = run_bass_kernel_spmd(_get_nc(), in_maps, core_ids=list(range(N_CORES)))
    # host epilogue of the reduction: per-core constant loss-weight scaling
    # (0.5*LOSS_W[k], weight counted once via the k==0 cores) + all-reduce
    loss = np.float32(0.0)
    weight = np.float32(0.0)
    for c in range(N_CORES):
        k = c % 4
        p6 = res.results[c]["partial"][0]
        loss += np.float32(p6[0:3].sum() * np.float32(0.5 * LOSS_W[k]))
        if k == 0:
            weight += np.float32(p6[3:6].sum())
    return (np.array([loss], np.float32), np.array([weight], np.float32))



# revision 5
# speedup vs baseline: 1.2484x; 1.2484x over previous
"""Bbox regression loss (smooth-L1 over gathered bbox deltas) on 8 TRN2 cores.

The loss gathers 4 scalars per (batch, gt-box) from each FPN level's dense
prediction tensor, applies smooth-L1 against the gt deltas, and reduces to
two scalars (weighted loss sum, valid-box count).  Only 3 x 2 x 128 x 4 =
3072 elements of the ~92MB of predictions are ever read, so the kernel is
built around one on-device dma_gather rather than streaming.

Sharding: core c handles (b = c//4, k = c%4) where k indexes the 4 bbox
coordinate channels (channel group k*A:(k+1)*A of the 4*A=12 channel dim).
Each core receives exactly 1/8 of every prediction tensor (concatenated
into one row table), computes its partial (loss, weight) fully on device,
and the host sums the 8 partials.

Device pipeline per core (critical path = 3 chained DMAs, everything else
is hidden):
  1. aux load via a PREPARE_ONLY SWDGE gather with static iota indices:
     the descriptor-gen runs before the program's start barrier and the
     trigger fires immediately, skipping the HWDGE + DGE-delay fixed costs
     of a regular dma_start.  The aux row per gt-entry carries gt deltas,
     validity, in-row element offsets (rem) and the packed int16 gather row
     indices -- all precomputed on host from the (small) coord tensors.
     Masked entries (pad gt or inactive sample) are pointed at a zero pad
     row appended to the table with gt=0, so they contribute exactly 0 loss
     with no on-device masking.
  2. main dma_gather (PREPARE_ONLY + trigger) fetches 384 512B rows from
     the concatenated prediction table -> g[m, level, 128] f32.
  3. fused scalar_tensor_tensor one-hot select (iota==rem)*g with
     per-partition accumulate -> pred[m,l]; smooth-L1 via the identity
     2*sl(d) = min(|d|,1) * max(2|d|-1, |d|) (the 0.5 folded into the
     host-side loss weight); result written next to the validity columns.
  4. output via a PREPARE_ONLY dma_scatter_add whose 128 indices all hit
     row 0 of the (pre-zeroed) output: the DMA engine itself performs the
     partition reduction, replacing the PE matmul + PSUM copy + HWDGE
     output DMA with a single trigger fired right after the last vector op.
"""

import os

import numpy as np

try:  # persistent XLA/NEFF compile cache across processes
    import jax

    os.makedirs("/tmp/jax_pcache", exist_ok=True)
    jax.config.update("jax_compilation_cache_dir", "/tmp/jax_pcache")
    jax.config.update("jax_persistent_cache_min_compile_time_secs", 0.0)
    jax.config.update("jax_persistent_cache_min_entry_size_bytes", 0)
except Exception:
    pass

import concourse.bacc as bacc
import concourse.bass as bass
import concourse.tile as tile
from concourse import mybir
from concourse.bass_utils import run_bass_kernel_spmd

A = 3                       # anchors per level
M = 128                     # gt entries per sample
GRIDS = (96, 48, 24)        # level l grid; level l uses coord/diff index 2-l
LOSS_W = (1.0, 1.0, 1.0, 0.1)
ROW = 128                   # f32 elements per gather row (512B)
NLVL = 3
NIDX = NLVL * M             # 384 gathered rows per core
V = tuple(A * g * g * g // ROW for g in GRIDS)      # (20736, 2592, 324)
VBASE = (0, V[0], V[0] + V[1])
VTOT = sum(V)               # 23652 rows; +1 zero pad row < int16 max
N_CORES = 8

AUXC = 64                   # aux row: 256B gather granularity
# aux f32 columns: 0:3 gt | 3:6 validf | 6:9 remf | 10:22 idx16 (bitcast)
IDXC = 10

F32 = mybir.dt.float32
I16 = mybir.dt.int16
Alu = mybir.AluOpType


def _build_bass() -> bass.Bass:
    nc = bacc.Bacc(
        "TRN2",
        target_bir_lowering=False,
        debug=False,
        num_devices=N_CORES,
        num_swdge_queues=3,
    )
    tab = nc.dram_tensor("tab", [VTOT + 1, ROW], F32, kind="ExternalInput")
    auxd = nc.dram_tensor("aux", [M, AUXC], F32, kind="ExternalInput")
    out = nc.dram_tensor("partial", [1, AUXC], F32, kind="ExternalOutput")

    with tile.TileContext(nc) as tc:
        with tc.tile_pool(name="sb", bufs=1) as sb:
            aux = sb.tile([M, AUXC], F32)
            g = sb.tile([M, NLVL, ROW], F32)
            io = sb.tile([M, ROW], F32)
            zi = sb.tile([M, NIDX // 16 // 3], I16)   # [128, 8] zeros
            aipre = sb.tile([M, M // 16], I16)        # [128, 8]
            pcol = sb.tile([M, 1], I16)
            pm16 = sb.tile([M, 1], I16)
            ai = sb.tile([M, M // 16], I16)
            pred = sb.tile([M, NLVL], F32)
            scr0 = sb.tile([M, ROW], F32)
            scr1 = sb.tile([M, ROW], F32)
            d = sb.tile([M, NLVL], F32)
            ad = sb.tile([M, NLVL], F32)
            pmin = sb.tile([M, NLVL], F32)
            r = sb.tile([M, NLVL], F32)
            q = sb.tile([M, NLVL], F32)

            # --- static index material, runs before the start barrier ---
            nc.gpsimd.memset(zi[:], 0)
            # aux gather idx, wrapped+replicated: ai[p, c] = 16*c + p%16
            nc.gpsimd.iota(aipre[:], [[16, M // 16]], channel_multiplier=0)
            nc.gpsimd.iota(pcol[:], [[0, 1]], channel_multiplier=1)
            with nc.allow_low_precision(reason="exact small-int index math"):
                nc.gpsimd.tensor_scalar(
                    pm16[:], pcol[:], 15, None, Alu.bitwise_and
                )
                # aipre is a multiple of 16 and pm16 in [0,16): OR == ADD
                nc.gpsimd.tensor_scalar(
                    ai[:], aipre[:], pm16[:, 0:1], None, Alu.bitwise_or
                )
            nc.gpsimd.iota(
                io[:],
                [[1, ROW]],
                channel_multiplier=0,
                allow_small_or_imprecise_dtypes=True,
            )

            aux3 = aux[:].rearrange("p (a f) -> p a f", a=1)
            # --- aux load: prep early, trigger fires at program start ---
            nc.gpsimd.dma_gather(
                aux3, auxd[:], ai[:], M, M, AUXC,
                prepare_only=True, queue_num=0,
                sem=nc.alloc_semaphore("aux_dma"),
            )
            nc.gpsimd.trigger_dma(count=None, queue_num=0)

            # --- main gather: 384 rows of 512B; prep waits only on aux ---
            idx16 = aux[:, IDXC : IDXC + NIDX // 16 // 2].bitcast(I16)
            nc.gpsimd.dma_gather(
                g[:], tab[:], idx16, NIDX, NIDX, ROW,
                prepare_only=True, queue_num=1,
                sem=nc.alloc_semaphore("g_dma"),
            )
            nc.gpsimd.trigger_dma(count=None, queue_num=1)

            # --- output scatter-add: all 128 idx hit row 0 (the DMA is the
            # partition reduction); prep now, trigger after the last op ---
            nc.gpsimd.dma_scatter_add(
                out[:], aux3, zi[:], M, M, AUXC,
                prepare_only=True, queue_num=2,
                sem=nc.alloc_semaphore("out_dma"),
            )

            # pred[m,l] = g[m,l,rem[m,l]] -- fused (iota==rem)*g + row-sum
            gts = aux[:, 0:3]
            remf = aux[:, 6:9]
            for lvl, eng, scr in ((0, nc.vector, scr0), (1, nc.gpsimd, scr1), (2, nc.vector, scr0)):
                eng.scalar_tensor_tensor(
                    out=scr[:],
                    in0=io[:],
                    scalar=remf[:, lvl : lvl + 1],
                    in1=g[:, lvl, :],
                    op0=Alu.is_equal,
                    op1=Alu.mult,
                    accum_out=pred[:, lvl : lvl + 1],
                )

            # smooth l1 (x2): min(|d|,1) * max(2|d|-1, |d|)  (0.5 in host wk)
            nc.vector.tensor_tensor(d[:], pred[:], gts, Alu.subtract)
            nc.vector.tensor_scalar(ad[:], d[:], 0.0, None, Alu.abs_max)
            nc.vector.tensor_scalar(pmin[:], ad[:], 1.0, None, Alu.min)
            nc.vector.tensor_scalar(r[:], ad[:], 2.0, 1.0, Alu.mult, Alu.subtract)
            nc.vector.tensor_tensor(q[:], r[:], ad[:], Alu.max)
            # sl2 lands in aux[:,0:3], next to validf in 3:6; junk in the
            # remaining columns is summed into out[0, 6:] which is unread.
            nc.vector.tensor_tensor(aux[:, 0:3], pmin[:], q[:], Alu.mult)
            nc.gpsimd.trigger_dma(count=None, queue_num=2)

    # Tile assigns each DMA a DMASW lane tick and points every consumer wait
    # at the lane semaphore, but for PREPARE_ONLY preps it leaves the user
    # `sem=` as on_update[0] (the slot both hardware SDMA and the sim bump on
    # DMA completion).  Repoint on_update[0] at the lane semaphore so the
    # completion actually satisfies the consumers.
    from concourse.tile_scheduler import PROC_NAMES

    fn = nc.m.functions[0]
    lane_sem: dict[str, tuple[int, str]] = {}
    for bb in fn.blocks:
        for ins in bb.instructions:
            si = ins.sync_info
            if si is None:
                continue
            for w in si.on_wait:
                if w.ant_name and w.ant_name.startswith("DMASW"):
                    lane_sem[w.ant_name.split("_")[0]] = (w.id, w.ant_name)
    for bb in fn.blocks:
        for ins in bb.instructions:
            if getattr(ins, "gen_mode", 0) != 1:
                continue
            lane = PROC_NAMES[ins.bass_scheduled_proc]
            assert lane.startswith("DMASW"), lane
            sem_id, sem_name = lane_sem[lane]
            u0 = ins.sync_info.on_update[0]
            u0.id = sem_id
            u0.ant_name = sem_name
    nc.finalize()
    return nc


_NC = None


def _get_nc():
    global _NC
    if _NC is None:
        _NC = _build_bass()
    return _NC


def kernel(**inputs: np.ndarray):
    out_l = [np.asarray(inputs[n]) for n in ("out1", "out3", "out5")]
    # level l uses coord/diff (2-l)  (the reference pairs them reversed)
    coords = [np.asarray(inputs[f"coord{2 - l}"]) for l in range(3)]
    diffs = [np.asarray(inputs[f"diff{2 - l}"]) for l in range(3)]

    in_maps = []
    for c in range(N_CORES):
        b, k = c // 4, c % 4
        im = {}
        im["tab"] = np.concatenate(
            [
                np.ascontiguousarray(out_l[l][b, A * k : A * (k + 1)]).reshape(
                    V[l], ROW
                )
                for l in range(3)
            ]
            + [np.zeros((1, ROW), np.float32)],
            axis=0,
        )
        aux = np.zeros((M, AUXC), np.float32)
        rows = np.zeros((M, NLVL), np.int64)
        for l, g in enumerate(GRIDS):
            cc = coords[l][b].astype(np.int64)  # [128, 4]
            valid = cc[:, 0] > -1
            active = bool(cc[0, 0] > -1)
            mask = valid & active
            a = np.maximum(cc[:, 0], 0)
            flat = ((a * g + cc[:, 1]) * g + cc[:, 2]) * g + cc[:, 3]
            row = VBASE[l] + (flat >> 7)
            rem = flat & (ROW - 1)
            row[~mask] = VTOT          # zero pad row
            rem[~mask] = 0
            rows[:, l] = row
            aux[:, 6 + l] = rem.astype(np.float32)
            aux[:, 3 + l] = mask.astype(np.float32)
            aux[mask, l] = diffs[l][b, mask, k]
        # wrapped idx layout: item i = l*128 + m -> idxw[i%16, i//16],
        # replicated across the 8 gpsimd cores (partition p reads p%16 row)
        idxw = np.zeros((16, NIDX // 16), np.int16)
        for l in range(NLVL):
            for mm in range(M):
                i = l * M + mm
                idxw[i % 16, i // 16] = rows[mm, l]
        aux[:, IDXC : IDXC + NIDX // 16 // 2] = np.tile(idxw, (8, 1)).view(
            np.float32
        )
        im["aux"] = aux
        in_maps.append(im)

    res = run_bass_kernel_spmd(_get_nc(), in_maps, core_ids=list(range(N_CORES)))
    # host epilogue of the reduction: per-core constant loss-weight scaling
    # (0.5*LOSS_W[k], weight counted once via the k==0 cores) + all-reduce
    loss = np.float32(0.0)
    weight = np.float32(0.0)
    for c in range(N_CORES):
        k = c % 4
        p6 = res.results[c]["partial"][0]
        loss += np.float32(p6[0:3].sum() * np.float32(0.5 * LOSS_W[k]))
        if k == 0:
            weight += np.float32(p6[3:6].sum())
    return (np.array([loss], np.float32), np.array([weight], np.float32))


# revision 6
# speedup vs baseline: 1.2980x; 1.0398x over previous
"""Bbox regression loss (smooth-L1 over gathered bbox deltas) on 8 TRN2 cores.

The loss gathers 4 scalars per (batch, gt-box) from each FPN level's dense
prediction tensor, applies smooth-L1 against the gt deltas, and reduces to
two scalars (weighted loss sum, valid-box count).  Only 3 x 2 x 128 x 4 =
3072 elements of the ~92MB of predictions are ever read, so the kernel is
built around one on-device dma_gather rather than streaming.

Sharding: core c handles (b = c//4, k = c%4) where k indexes the 4 bbox
coordinate channels (channel group k*A:(k+1)*A of the 4*A=12 channel dim).
Each core receives exactly 1/8 of every prediction tensor (concatenated
into one row table), computes its partial (loss, weight) fully on device,
and the host sums the 8 partials.

Device pipeline per core (critical path = 3 chained DMAs, everything else
is hidden):
  1. aux load via a PREPARE_ONLY SWDGE gather with static iota indices:
     the descriptor-gen runs before the program's start barrier and the
     trigger fires immediately, skipping the HWDGE + DGE-delay fixed costs
     of a regular dma_start.  The aux row per gt-entry carries gt deltas,
     validity, in-row element offsets (rem) and the packed int16 gather row
     indices -- all precomputed on host from the (small) coord tensors.
     Masked entries (pad gt or inactive sample) are pointed at a zero pad
     row appended to the table with gt=0, so they contribute exactly 0 loss
     with no on-device masking.
  2. main dma_gather (PREPARE_ONLY + trigger) fetches 384 512B rows from
     the concatenated prediction table -> g[m, level, 128] f32.
  3. fused scalar_tensor_tensor one-hot select (iota==rem)*g with
     per-partition accumulate -> pred[m,l]; smooth-L1 via the identity
     2*sl(d) = min(|d|,1) * max(2|d|-1, |d|) (the 0.5 folded into the
     host-side loss weight); result written next to the validity columns.
  4. output via a PREPARE_ONLY dma_scatter_add whose 128 indices all hit
     row 0 of the (pre-zeroed) output: the DMA engine itself performs the
     partition reduction, replacing the PE matmul + PSUM copy + HWDGE
     output DMA with a single trigger fired right after the last vector op.
"""

import os

import numpy as np

try:  # persistent XLA/NEFF compile cache across processes
    import jax

    os.makedirs("/tmp/jax_pcache", exist_ok=True)
    jax.config.update("jax_compilation_cache_dir", "/tmp/jax_pcache")
    jax.config.update("jax_persistent_cache_min_compile_time_secs", 0.0)
    jax.config.update("jax_persistent_cache_min_entry_size_bytes", 0)
except Exception:
    pass

import concourse.bacc as bacc
import concourse.bass as bass
import concourse.tile as tile
from concourse import mybir
from concourse.bass_utils import run_bass_kernel_spmd

A = 3                       # anchors per level
M = 128                     # gt entries per sample
GRIDS = (96, 48, 24)        # level l grid; level l uses coord/diff index 2-l
LOSS_W = (1.0, 1.0, 1.0, 0.1)
ROW = 128                   # f32 elements per gather row (512B)
NLVL = 3
NIDX = NLVL * M             # 384 gathered rows per core
V = tuple(A * g * g * g // ROW for g in GRIDS)      # (20736, 2592, 324)
VBASE = (0, V[0], V[0] + V[1])
VTOT = sum(V)               # 23652 rows; +1 zero pad row < int16 max
N_CORES = 8

AUXC = 64                   # aux row: 256B gather granularity
# aux f32 columns: 0:3 gt | 3:6 validf | 6:9 remf | 10:22 idx16 (bitcast)
IDXC = 10

F32 = mybir.dt.float32
I16 = mybir.dt.int16
Alu = mybir.AluOpType


def _build_bass() -> bass.Bass:
    nc = bacc.Bacc(
        "TRN2",
        target_bir_lowering=False,
        debug=False,
        num_devices=N_CORES,
        num_swdge_queues=3,
    )
    tab = nc.dram_tensor("tab", [VTOT + 1, ROW], F32, kind="ExternalInput")
    auxd = nc.dram_tensor("aux", [M, AUXC], F32, kind="ExternalInput")
    out = nc.dram_tensor("partial", [1, AUXC], F32, kind="ExternalOutput")

    with tile.TileContext(nc) as tc:
        with tc.tile_pool(name="sb", bufs=1) as sb:
            aux = sb.tile([M, AUXC], F32)
            g = sb.tile([M, NLVL, ROW], F32)
            io = sb.tile([M, ROW], F32)
            zi = sb.tile([M, NIDX // 16 // 3], I16)   # [128, 8] zeros
            aipre = sb.tile([M, M // 16], I16)        # [128, 8]
            pcol = sb.tile([M, 1], I16)
            ai = sb.tile([M, M // 16], I16)
            pred = sb.tile([M, NLVL], F32)
            scr0 = sb.tile([M, ROW], F32)
            scr1 = sb.tile([M, ROW], F32)
            d = sb.tile([M, NLVL], F32)
            pmin = sb.tile([M, NLVL], F32)
            t1 = sb.tile([M, NLVL], F32)
            q = sb.tile([M, NLVL], F32)

            # --- aux gather idx (the only dep of the first prep, so it is
            # built first): wrapped+replicated ai[p, c] = 16*c + p%16 ---
            nc.gpsimd.iota(aipre[:], [[16, M // 16]], channel_multiplier=0)
            nc.gpsimd.iota(pcol[:], [[0, 1]], channel_multiplier=1)
            with nc.allow_low_precision(reason="exact small-int index math"):
                # aipre is a multiple of 16 and p%16 in [0,16): OR == ADD
                nc.gpsimd.scalar_tensor_tensor(
                    out=ai[:],
                    in0=pcol[:].broadcast_to([M, M // 16]),
                    scalar=15,
                    in1=aipre[:],
                    op0=Alu.bitwise_and,
                    op1=Alu.bitwise_or,
                )

            aux3 = aux[:].rearrange("p (a f) -> p a f", a=1)
            # --- aux load: prep early, trigger fires at program start ---
            nc.gpsimd.dma_gather(
                aux3, auxd[:], ai[:], M, M, AUXC,
                prepare_only=True, queue_num=0,
                sem=nc.alloc_semaphore("aux_dma"),
            )
            nc.gpsimd.trigger_dma(count=None, queue_num=0)

            # static material not needed until after the main gather lands
            nc.gpsimd.memset(zi[:], 0)
            nc.gpsimd.iota(
                io[:],
                [[1, ROW]],
                channel_multiplier=0,
                allow_small_or_imprecise_dtypes=True,
            )

            # --- main gather: 384 rows of 512B; prep waits only on aux ---
            idx16 = aux[:, IDXC : IDXC + NIDX // 16 // 2].bitcast(I16)
            nc.gpsimd.dma_gather(
                g[:], tab[:], idx16, NIDX, NIDX, ROW,
                prepare_only=True, queue_num=1,
                sem=nc.alloc_semaphore("g_dma"),
            )
            nc.gpsimd.trigger_dma(count=None, queue_num=1)

            # --- output scatter-add: all 128 idx hit row 0 (the DMA is the
            # partition reduction); prep in the gather-transfer window,
            # trigger after the last vector op ---
            nc.gpsimd.dma_scatter_add(
                out[:], aux3, zi[:], M, M, AUXC,
                prepare_only=True, queue_num=2,
                sem=nc.alloc_semaphore("out_dma"),
            )

            # pred[m,l] = g[m,l,rem[m,l]] -- fused (iota==rem)*g + row-sum
            gts = aux[:, 0:3]
            remf = aux[:, 6:9]
            for lvl, eng, scr in (
                (1, nc.gpsimd, scr1),
                (0, nc.vector, scr0),
                (2, nc.vector, scr0),
            ):
                eng.scalar_tensor_tensor(
                    out=scr[:],
                    in0=io[:],
                    scalar=remf[:, lvl : lvl + 1],
                    in1=g[:, lvl, :],
                    op0=Alu.is_equal,
                    op1=Alu.mult,
                    accum_out=pred[:, lvl : lvl + 1],
                )

            # smooth l1 (x2) in 5 ops via
            #   2*sl(d) = (relu(|d|-1) + (|d|-1) + 1) * min(|d|,1)
            # (|d|<1: |d|*|d|; |d|>=1: (2|d|-1)*1; the 0.5 in host wk)
            nc.vector.tensor_tensor(d[:], pred[:], gts, Alu.subtract)
            nc.vector.tensor_scalar(t1[:], d[:], 0.0, 1.0, Alu.abs_max, Alu.subtract)
            nc.vector.scalar_tensor_tensor(
                out=q[:], in0=t1[:], scalar=0.0, in1=t1[:],
                op0=Alu.max, op1=Alu.add,
            )
            nc.vector.tensor_scalar(pmin[:], d[:], 0.0, 1.0, Alu.abs_max, Alu.min)
            # sl2 lands in aux[:,0:3], next to validf in 3:6; junk in the
            # remaining columns is summed into out[0, 6:] which is unread.
            nc.vector.scalar_tensor_tensor(
                out=aux[:, 0:3], in0=q[:], scalar=1.0, in1=pmin[:],
                op0=Alu.add, op1=Alu.mult,
            )
            nc.gpsimd.trigger_dma(count=None, queue_num=2)

    # Tile assigns each DMA a DMASW lane tick and points every consumer wait
    # at the lane semaphore, but for PREPARE_ONLY preps it leaves the user
    # `sem=` as on_update[0] (the slot both hardware SDMA and the sim bump on
    # DMA completion).  Repoint on_update[0] at the lane semaphore so the
    # completion actually satisfies the consumers.
    from concourse.tile_scheduler import PROC_NAMES

    fn = nc.m.functions[0]
    lane_sem: dict[str, tuple[int, str]] = {}
    for bb in fn.blocks:
        for ins in bb.instructions:
            si = ins.sync_info
            if si is None:
                continue
            for w in si.on_wait:
                if w.ant_name and w.ant_name.startswith("DMASW"):
                    lane_sem[w.ant_name.split("_")[0]] = (w.id, w.ant_name)
    for bb in fn.blocks:
        for ins in bb.instructions:
            if getattr(ins, "gen_mode", 0) != 1:
                continue
            lane = PROC_NAMES[ins.bass_scheduled_proc]
            assert lane.startswith("DMASW"), lane
            sem_id, sem_name = lane_sem[lane]
            u0 = ins.sync_info.on_update[0]
            u0.id = sem_id
            u0.ant_name = sem_name
    nc.finalize()
    return nc


_NC = None


def _get_nc():
    global _NC
    if _NC is None:
        _NC = _build_bass()
    return _NC


def kernel(**inputs: np.ndarray):
    out_l = [np.asarray(inputs[n]) for n in ("out1", "out3", "out5")]
    # level l uses coord/diff (2-l)  (the reference pairs them reversed)
    coords = [np.asarray(inputs[f"coord{2 - l}"]) for l in range(3)]
    diffs = [np.asarray(inputs[f"diff{2 - l}"]) for l in range(3)]

    in_maps = []
    for c in range(N_CORES):
        b, k = c // 4, c % 4
        im = {}
        im["tab"] = np.concatenate(
            [
                np.ascontiguousarray(out_l[l][b, A * k : A * (k + 1)]).reshape(
                    V[l], ROW
                )
                for l in range(3)
            ]
            + [np.zeros((1, ROW), np.float32)],
            axis=0,
        )
        aux = np.zeros((M, AUXC), np.float32)
        rows = np.zeros((M, NLVL), np.int64)
        for l, g in enumerate(GRIDS):
            cc = coords[l][b].astype(np.int64)  # [128, 4]
            valid = cc[:, 0] > -1
            active = bool(cc[0, 0] > -1)
            mask = valid & active
            a = np.maximum(cc[:, 0], 0)
            flat = ((a * g + cc[:, 1]) * g + cc[:, 2]) * g + cc[:, 3]
            row = VBASE[l] + (flat >> 7)
            rem = flat & (ROW - 1)
            row[~mask] = VTOT          # zero pad row
            rem[~mask] = 0
            rows[:, l] = row
            aux[:, 6 + l] = rem.astype(np.float32)
            aux[:, 3 + l] = mask.astype(np.float32)
            aux[mask, l] = diffs[l][b, mask, k]
        # wrapped idx layout: item i = l*128 + m -> idxw[i%16, i//16],
        # replicated across the 8 gpsimd cores (partition p reads p%16 row)
        idxw = np.zeros((16, NIDX // 16), np.int16)
        for l in range(NLVL):
            for mm in range(M):
                i = l * M + mm
                idxw[i % 16, i // 16] = rows[mm, l]
        aux[:, IDXC : IDXC + NIDX // 16 // 2] = np.tile(idxw, (8, 1)).view(
            np.float32
        )
        im["aux"] = aux
        in_maps.append(im)

    res = run_bass_kernel_spmd(_get_nc(), in_maps, core_ids=list(range(N_CORES)))
    # host epilogue of the reduction: per-core constant loss-weight scaling
    # (0.5*LOSS_W[k], weight counted once via the k==0 cores) + all-reduce
    loss = np.float32(0.0)
    weight = np.float32(0.0)
    for c in range(N_CORES):
        k = c % 4
        p6 = res.results[c]["partial"][0]
        loss += np.float32(p6[0:3].sum() * np.float32(0.5 * LOSS_W[k]))
        if k == 0:
            weight += np.float32(p6[3:6].sum())
    return (np.array([loss], np.float32), np.array([weight], np.float32))


# revision 9
# speedup vs baseline: 1.4003x; 1.0788x over previous
"""Bbox regression loss (smooth-L1 over gathered bbox deltas) on 8 TRN2 cores.

The loss gathers 4 scalars per (batch, gt-box) from each FPN level's dense
prediction tensor, applies smooth-L1 against the gt deltas, and reduces to
two scalars (weighted loss sum, valid-box count).  Only 3 x 2 x 128 x 4 =
3072 elements of the ~92MB of predictions are ever read, so the kernel is
built around one on-device dma_gather rather than streaming.

Sharding: core c handles (b = c//4, k = c%4) where k indexes the 4 bbox
coordinate channels (channel group k*A:(k+1)*A of the 4*A=12 channel dim).
Each core receives exactly 1/8 of every prediction tensor (concatenated
into one row table), computes its partial (loss, weight) fully on device,
and the host sums the 8 partials.

Device pipeline per core (critical path = 3 chained DMAs, everything else
is hidden):
  1. aux load via a PREPARE_ONLY SWDGE gather with static iota indices:
     the descriptor-gen runs before the program's start barrier and the
     trigger fires immediately, skipping the HWDGE + DGE-delay fixed costs
     of a regular dma_start.  The aux row per gt-entry carries gt deltas,
     validity, in-row element offsets (rem) and the packed int16 gather row
     indices -- all precomputed on host from the (small) coord tensors.
     Masked entries (pad gt or inactive sample) are pointed at a zero pad
     row appended to the table with gt=0, so they contribute exactly 0 loss
     with no on-device masking.
  2. main dma_gather (PREPARE_ONLY + trigger) fetches 384 512B rows from
     the concatenated prediction table -> g[m, level, 128] f32.
  3. fused scalar_tensor_tensor one-hot select (iota==rem)*g with
     per-partition accumulate -> pred[m,l]; smooth-L1 via the identity
     2*sl(d) = min(|d|,1) * max(2|d|-1, |d|) (the 0.5 folded into the
     host-side loss weight); result written next to the validity columns.
  4. output via a PREPARE_ONLY dma_scatter_add whose 128 indices all hit
     row 0 of the (pre-zeroed) output: the DMA engine itself performs the
     partition reduction, replacing the PE matmul + PSUM copy + HWDGE
     output DMA with a single trigger fired right after the last vector op.
"""

import os

import numpy as np

try:  # persistent XLA/NEFF compile cache across processes
    import jax

    os.makedirs("/tmp/jax_pcache", exist_ok=True)
    jax.config.update("jax_compilation_cache_dir", "/tmp/jax_pcache")
    jax.config.update("jax_persistent_cache_min_compile_time_secs", 0.0)
    jax.config.update("jax_persistent_cache_min_entry_size_bytes", 0)
except Exception:
    pass

import concourse.bacc as bacc
import concourse.bass as bass
import concourse.tile as tile
from concourse import mybir
from concourse.bass_utils import run_bass_kernel_spmd

A = 3                       # anchors per level
M = 128                     # gt entries per sample
GRIDS = (96, 48, 24)        # level l grid; level l uses coord/diff index 2-l
LOSS_W = (1.0, 1.0, 1.0, 0.1)
ROW = 128                   # f32 elements per gather row (512B)
NLVL = 3
NIDX = NLVL * M             # 384 gathered rows per core
V = tuple(A * g * g * g // ROW for g in GRIDS)      # (20736, 2592, 324)
VBASE = (0, V[0], V[0] + V[1])
VTOT = sum(V)               # 23652 rows; +1 zero pad row < int16 max
N_CORES = 8

AUXC = 64                   # aux row: 256B gather granularity
# aux f32 columns: 0:3 gt | 3:6 validf | 6:9 remf | 10:22 idx16 (bitcast)
IDXC = 10

F32 = mybir.dt.float32
I16 = mybir.dt.int16
Alu = mybir.AluOpType


def _build_bass() -> bass.Bass:
    nc = bacc.Bacc(
        "TRN2",
        target_bir_lowering=False,
        debug=False,
        num_devices=N_CORES,
        num_swdge_queues=3,
    )
    tab = nc.dram_tensor("tab", [VTOT + 1, ROW], F32, kind="ExternalInput")
    auxi = nc.dram_tensor("auxi", [M, 16], F32, kind="ExternalInput")
    auxd = nc.dram_tensor("aux", [M, AUXC], F32, kind="ExternalInput")
    out = nc.dram_tensor("partial", [1, AUXC], F32, kind="ExternalOutput")

    with tile.TileContext(nc) as tc:
        with tc.tile_pool(name="sb", bufs=1) as sb:
            aux = sb.tile([M, AUXC], F32)
            auxit = sb.tile([M, 16], F32)
            g = sb.tile([M, NLVL, ROW], F32)
            io = sb.tile([M, ROW], F32)
            ones = sb.tile([M, ROW], F32)
            zi = sb.tile([M, NIDX // 16 // 3], I16)   # [128, 8] zeros
            pred = sb.tile([M, NLVL], F32)
            scr0 = sb.tile([M, ROW], F32)
            scr1 = sb.tile([M, ROW], F32)
            d = sb.tile([M, NLVL], F32)
            pmin = sb.tile([M, NLVL], F32)
            t1 = sb.tile([M, NLVL], F32)
            q = sb.tile([M, NLVL], F32)

            # --- aux loads via HWDGE from SP: for the head-of-program DMA
            # (no waits) SEQ+HWDGE gen overlap the start barrier, beating a
            # SWDGE prep+trigger.  The 48B idx payload goes first/alone so
            # the gather prep can start ~125ns earlier. ---
            nc.sync.dma_start(out=auxit[:], in_=auxi[:])
            nc.sync.dma_start(out=aux[:], in_=auxd[:])

            # constants on the (otherwise idle) DVE: io = iota via prefix
            # scan of ones, zi = the scatter's all-zero index block
            nc.vector.memset(ones[:], 1.0)
            nc.vector.tensor_tensor_scan(
                io[:], ones[:], ones[:], -1.0, Alu.add, Alu.bypass
            )
            nc.vector.memset(zi[:], 0)

            # --- main gather: 384 rows of 512B; prep waits only on auxi ---
            idx16 = auxit[:, 0 : NIDX // 16 // 2].bitcast(I16)
            nc.gpsimd.dma_gather(
                g[:], tab[:], idx16, NIDX, NIDX, ROW,
                prepare_only=True, queue_num=0,
                sem=nc.alloc_semaphore("g_dma"),
            )
            nc.gpsimd.trigger_dma(count=None, queue_num=0)

            # --- output scatter-add: all 128 idx hit row 0 (the DMA is the
            # partition reduction); prep in the gather-transfer window,
            # trigger after the last vector op ---
            aux3 = aux[:].rearrange("p (a f) -> p a f", a=1)
            nc.gpsimd.dma_scatter_add(
                out[:], aux3, zi[:], M, M, AUXC,
                prepare_only=True, queue_num=1,
                sem=nc.alloc_semaphore("out_dma"),
            )

            # pred[m,l] = g[m,l,rem[m,l]] -- fused (iota==rem)*g + row-sum
            # Pool takes the last-consumed level so DVE's chain starts as
            # soon as its own two stts retire.
            gts = aux[:, 0:3]
            remf = aux[:, 6:9]
            for lvl, eng, scr in (
                (2, nc.gpsimd, scr1),
                (0, nc.vector, scr0),
                (1, nc.vector, scr0),
            ):
                eng.scalar_tensor_tensor(
                    out=scr[:],
                    in0=io[:],
                    scalar=remf[:, lvl : lvl + 1],
                    in1=g[:, lvl, :],
                    op0=Alu.is_equal,
                    op1=Alu.mult,
                    accum_out=pred[:, lvl : lvl + 1],
                )

            # smooth l1 (x2) in 5 ops via
            #   2*sl(d) = (relu(|d|-1) + (|d|-1) + 1) * min(|d|,1)
            # (|d|<1: |d|*|d|; |d|>=1: (2|d|-1)*1; the 0.5 in host wk)
            nc.vector.tensor_tensor(d[:], pred[:], gts, Alu.subtract)
            nc.vector.tensor_scalar(t1[:], d[:], 0.0, 1.0, Alu.abs_max, Alu.subtract)
            nc.vector.scalar_tensor_tensor(
                out=q[:], in0=t1[:], scalar=0.0, in1=t1[:],
                op0=Alu.max, op1=Alu.add,
            )
            nc.vector.tensor_scalar(pmin[:], d[:], 0.0, 1.0, Alu.abs_max, Alu.min)
            # sl2 lands in aux[:,0:3], next to validf in 3:6; junk in the
            # remaining columns is summed into out[0, 6:] which is unread.
            nc.vector.scalar_tensor_tensor(
                out=aux[:, 0:3], in0=q[:], scalar=1.0, in1=pmin[:],
                op0=Alu.add, op1=Alu.mult,
            )
            nc.gpsimd.trigger_dma(count=None, queue_num=1)

    # Tile assigns each DMA a DMASW lane tick and points every consumer wait
    # at the lane semaphore, but for PREPARE_ONLY preps it leaves the user
    # `sem=` as on_update[0] (the slot both hardware SDMA and the sim bump on
    # DMA completion).  Repoint on_update[0] at the lane semaphore so the
    # completion actually satisfies the consumers.
    from concourse.tile_scheduler import PROC_NAMES

    fn = nc.m.functions[0]
    lane_sem: dict[str, tuple[int, str]] = {}
    for bb in fn.blocks:
        for ins in bb.instructions:
            si = ins.sync_info
            if si is None:
                continue
            for w in si.on_wait:
                if w.ant_name and w.ant_name.startswith("DMASW"):
                    lane_sem[w.ant_name.split("_")[0]] = (w.id, w.ant_name)
    for bb in fn.blocks:
        for ins in bb.instructions:
            if getattr(ins, "gen_mode", 0) != 1:
                continue
            lane = PROC_NAMES[ins.bass_scheduled_proc]
            assert lane.startswith("DMASW"), lane
            sem_id, sem_name = lane_sem[lane]
            u0 = ins.sync_info.on_update[0]
            u0.id = sem_id
            u0.ant_name = sem_name
    nc.finalize()
    return nc


_NC = None


def _get_nc():
    global _NC
    if _NC is None:
        _NC = _build_bass()
    return _NC


def kernel(**inputs: np.ndarray):
    out_l = [np.asarray(inputs[n]) for n in ("out1", "out3", "out5")]
    # level l uses coord/diff (2-l)  (the reference pairs them reversed)
    coords = [np.asarray(inputs[f"coord{2 - l}"]) for l in range(3)]
    diffs = [np.asarray(inputs[f"diff{2 - l}"]) for l in range(3)]

    in_maps = []
    for c in range(N_CORES):
        b, k = c // 4, c % 4
        im = {}
        im["tab"] = np.concatenate(
            [
                np.ascontiguousarray(out_l[l][b, A * k : A * (k + 1)]).reshape(
                    V[l], ROW
                )
                for l in range(3)
            ]
            + [np.zeros((1, ROW), np.float32)],
            axis=0,
        )
        aux = np.zeros((M, AUXC), np.float32)
        rows = np.zeros((M, NLVL), np.int64)
        for l, g in enumerate(GRIDS):
            cc = coords[l][b].astype(np.int64)  # [128, 4]
            valid = cc[:, 0] > -1
            active = bool(cc[0, 0] > -1)
            mask = valid & active
            a = np.maximum(cc[:, 0], 0)
            flat = ((a * g + cc[:, 1]) * g + cc[:, 2]) * g + cc[:, 3]
            row = VBASE[l] + (flat >> 7)
            rem = flat & (ROW - 1)
            row[~mask] = VTOT          # zero pad row
            rem[~mask] = 0
            rows[:, l] = row
            aux[:, 6 + l] = rem.astype(np.float32)
            aux[:, 3 + l] = mask.astype(np.float32)
            aux[mask, l] = diffs[l][b, mask, k]
        # wrapped idx layout: item i = l*128 + m -> idxw[i%16, i//16],
        # replicated across the 8 gpsimd cores (partition p reads p%16 row)
        idxw = np.zeros((16, NIDX // 16), np.int16)
        for l in range(NLVL):
            for mm in range(M):
                i = l * M + mm
                idxw[i % 16, i // 16] = rows[mm, l]
        auxi = np.zeros((M, 16), np.float32)
        auxi[:, 0 : NIDX // 16 // 2] = np.tile(idxw, (8, 1)).view(np.float32)
        im["auxi"] = auxi
        im["aux"] = aux
        in_maps.append(im)

    res = run_bass_kernel_spmd(_get_nc(), in_maps, core_ids=list(range(N_CORES)))
    # host epilogue of the reduction: per-core constant loss-weight scaling
    # (0.5*LOSS_W[k], weight counted once via the k==0 cores) + all-reduce
    loss = np.float32(0.0)
    weight = np.float32(0.0)
    for c in range(N_CORES):
        k = c % 4
        p6 = res.results[c]["partial"][0]
        loss += np.float32(p6[0:3].sum() * np.float32(0.5 * LOSS_W[k]))
        if k == 0:
            weight += np.float32(p6[3:6].sum())
    return (np.array([loss], np.float32), np.array([weight], np.float32))
